# revision 10
# baseline (speedup 1.0000x reference)
"""HNetLoss Trainium2 kernel (raw Bass, manual sync).

Data-parallel over batch: 16384 samples on 8 NeuronCores (2048 each), processed
as 16 tiles of [128 partitions x 512 points] per core.

Math (homogeneous coord of gt_pts is 1 by construction):
  den = c5*y + 1 ; u = 1/den ; Y = (c3*y + c4)*u ; X = (c0*x + c1*y + c2)*u
  Cubic LS fit of X on [Y^3,Y^2,Y,1]: G w = rhs, Hankel G from S_k = sum Y^k,
  rhs_k = sum X*Y^(3-k).  H^-1 row 0 gives (a0,a1,a2); p = a0*w + [0,0,a1,a2].
  Per-sample loss*N = V - 2*p.U + p^T G p, U_k = sum x*Y^k, V = sum x^2.

Moments use fused accumulate ops spread across DVE / ACT / GPSIMD with manual
semaphore pipelining (double-buffered tiles). The batched 4x4 solves + loss
algebra run on DVE as [128,16]-wide ops. Output per core: [128,1] partial sums
(NaN-propagating, matching the f32 reference which is NaN for singular G);
the host sums the 8x128 partials (the trivial all-reduce) and divides by B*N.
"""

from contextlib import ExitStack

import numpy as np

import concourse.bass as bass
from concourse import mybir
from concourse.bass_utils import run_bass_kernel_spmd

F32 = mybir.dt.float32
AF = mybir.ActivationFunctionType
OP = mybir.AluOpType
AX = mybir.AxisListType

NCORES = 8
B, N = 16384, 512
BS = B // NCORES      # 2048 samples per core
P = 128
NT = BS // P          # 16 tiles per core

# moment slots (columns of macc, NT wide each):
#   S1..S6 -> 0..5 ; T0..T3 -> 6..9 ; U0..U3 -> 10..13 ; V -> 14
iS = lambda k: k - 1
iT = lambda k: 6 + k
iU = lambda k: 10 + k
iV = 14
NMOM = 15

NALG = 120  # [128,16] scratch slots for tail algebra (linear allocation)


def _build_kernel(NT=NT) -> bass.Bass:
    BS = NT * P
    nc = bass.Bass()
    gt = nc.declare_dram_parameter("gt_pts", [BS, N, 3], F32, isOutput=False)
    co = nc.declare_dram_parameter("transformation_coefficients", [BS, 6], F32, isOutput=False)
    out = nc.declare_dram_parameter("loss_part", [P, 1], F32, isOutput=True)

    gtv = gt[:].rearrange("(t p) n c -> t p n c", p=P)   # [NT, P, N, 3]
    cov = co[:].rearrange("(t p) j -> p t j", p=P)       # [P, NT, 6]

    with ExitStack() as ctx:
        e = ctx.enter_context

        inp = [e(nc.sbuf_tensor([P, N, 3], F32)) for _ in range(2)]
        num = [e(nc.sbuf_tensor([P, N], F32)) for _ in range(2)]
        t1b = [e(nc.sbuf_tensor([P, N], F32)) for _ in range(2)]
        t2b = [e(nc.sbuf_tensor([P, N], F32)) for _ in range(2)]
        den = [e(nc.sbuf_tensor([P, N], F32)) for _ in range(2)]
        ub = [e(nc.sbuf_tensor([P, N], F32)) for _ in range(2)]
        Yb = [e(nc.sbuf_tensor([P, N], F32)) for _ in range(2)]
        Xb = [e(nc.sbuf_tensor([P, N], F32)) for _ in range(2)]
        Y2b = [e(nc.sbuf_tensor([P, N], F32)) for _ in range(2)]
        Y3b = [e(nc.sbuf_tensor([P, N], F32)) for _ in range(2)]
        jD = e(nc.sbuf_tensor([P, N], F32))   # per-engine junk outputs
        jA = e(nc.sbuf_tensor([P, N], F32))
        jP = e(nc.sbuf_tensor([P, N], F32))
        csb = e(nc.sbuf_tensor([P, NT * 6], F32))
        macc = e(nc.sbuf_tensor([P, NMOM * NT], F32))
        alg = e(nc.sbuf_tensor([P, NALG * NT], F32))
        red = e(nc.sbuf_tensor([P, 1], F32))

        dma = e(nc.semaphore("dma"))
        aP_s = e(nc.semaphore("aP"))
        aQ_s = e(nc.semaphore("aQ"))
        aD_s = e(nc.semaphore("aD"))
        dY_s = e(nc.semaphore("dY"))
        dX_s = e(nc.semaphore("dX"))
        dZ_s = e(nc.semaphore("dZ"))
        dD_s = e(nc.semaphore("dD"))
        gT_s = e(nc.semaphore("gT"))
        gD_s = e(nc.semaphore("gD"))
        vT_s = e(nc.semaphore("vT"))

        csbv = csb[:].rearrange("p (t j) -> p t j", j=6)
        maccv = macc[:]

        def acc(m, t):
            return maccv[:, m * NT + t : m * NT + t + 1]

        def mom(m):
            return maccv[:, m * NT : (m + 1) * NT]

        def c(t, j):
            return csbv[:, t, j : j + 1]

        block = e(nc.Block())

        @block.sync
        def _(sync):
            sync.dma_start(out=csb[:], in_=cov).then_inc(dma, 16)
            for t in range(NT):
                if t >= 2:
                    sync.wait_ge(aD_s, t - 1)
                    sync.wait_ge(dD_s, t - 1)
                    sync.wait_ge(gD_s, t - 1)
                sync.dma_start(out=inp[t % 2][:], in_=gtv[t]).then_inc(dma, 16)
            sync.wait_ge(vT_s, 1)
            sync.dma_start(out=out[:], in_=red[:]).then_inc(dma, 16)

        @block.scalar
        def _(scalar):
            for t in range(NT):
                b = t % 2
                x = inp[b][:, :, 0]
                y = inp[b][:, :, 1]
                if t >= 2:
                    scalar.wait_ge(dD_s, t - 1)
                    scalar.wait_ge(gD_s, t - 1)
                scalar.wait_ge(dma, 16 * (t + 2))
                scalar.activation(num[b][:], y, AF.Identity, bias=c(t, 4), scale=c(t, 3))
                scalar.activation(t1b[b][:], x, AF.Identity, bias=c(t, 2),
                                  scale=c(t, 0)).then_inc(aP_s, 1)
                scalar.wait_ge(dY_s, t + 1)
                scalar.activation(Y2b[b][:], Yb[b][:], AF.Square,
                                  accum_out=acc(iS(2), t))
                scalar.activation(jA[:], Y2b[b][:], AF.Square,
                                  accum_out=acc(iS(4), t)).then_inc(aQ_s, 1)
                scalar.wait_ge(dZ_s, t + 1)
                scalar.activation(jA[:], Y3b[b][:], AF.Square, accum_out=acc(iS(6), t))
                scalar.activation(jA[:], x, AF.Square, accum_out=acc(iV, t))
                scalar.activation(jA[:], x, AF.Copy, accum_out=acc(iU(0), t))
                scalar.wait_ge(gD_s, t + 1)
                scalar.activation(jA[:], jP2[b][:], AF.Copy, accum_out=acc(iT(3), t))
                scalar.activation(jA[:], jP4[b][:], AF.Copy,
                                  accum_out=acc(iU(3), t)).then_inc(aD_s, 1)

        @block.gpsimd
        def _(gpsimd):
            for t in range(NT):
                b = t % 2
                x = inp[b][:, :, 0]
                y = inp[b][:, :, 1]
                if t >= 2:
                    gpsimd.wait_ge(aD_s, t - 1)
                    gpsimd.wait_ge(dD_s, t - 1)
                gpsimd.wait_ge(dma, 16 * (t + 2))
                gpsimd.wait_ge(aP_s, t + 1)
                gpsimd.tensor_scalar(jPt[:], y, c(t, 1), None, OP.mult)
                gpsimd.tensor_tensor(t2b[b][:], jPt[:], t1b[b][:],
                                     OP.add).then_inc(gT_s, 1)
                gpsimd.wait_ge(aQ_s, t + 1)
                gpsimd.tensor_tensor(jP3[b][:], x, Y2b[b][:], OP.mult)
                gpsimd.wait_ge(dZ_s, t + 1)
                gpsimd.tensor_tensor(jP1[b][:], Y2b[b][:], Y3b[b][:], OP.mult)
                gpsimd.tensor_tensor(jP4[b][:], x, Y3b[b][:], OP.mult)
                gpsimd.tensor_tensor(jP2[b][:], Xb[b][:], Y3b[b][:],
                                     OP.mult).then_inc(gD_s, 1)

        @block.vector
        def _(vector):
            for t in range(NT):
                b = t % 2
                x = inp[b][:, :, 0]
                y = inp[b][:, :, 1]
                if t >= 2:
                    vector.wait_ge(aD_s, t - 1)
                    vector.wait_ge(gD_s, t - 1)
                vector.wait_ge(dma, 16 * (t + 2))
                vector.tensor_scalar(den[b][:], y, c(t, 5), 1.0, OP.mult, OP.add)
                vector.reciprocal(ub[b][:], den[b][:])
                vector.wait_ge(aP_s, t + 1)
                vector.scalar_tensor_tensor(Yb[b][:], num[b][:], 1.0, ub[b][:],
                                            OP.bypass, OP.mult,
                                            accum_out=acc(iS(1), t)).then_inc(dY_s, 1)
                vector.wait_ge(gT_s, t + 1)
                vector.scalar_tensor_tensor(Xb[b][:], t2b[b][:], 1.0, ub[b][:],
                                            OP.bypass, OP.mult,
                                            accum_out=acc(iT(0), t)).then_inc(dX_s, 1)
                vector.wait_ge(aQ_s, t + 1)
                vector.scalar_tensor_tensor(Y3b[b][:], Y2b[b][:], 1.0, Yb[b][:],
                                            OP.bypass, OP.mult,
                                            accum_out=acc(iS(3), t)).then_inc(dZ_s, 1)
                vector.scalar_tensor_tensor(jD[:], Xb[b][:], 1.0, Yb[b][:], OP.bypass,
                                            OP.mult, accum_out=acc(iT(1), t))
                vector.scalar_tensor_tensor(jD[:], Xb[b][:], 1.0, Y2b[b][:], OP.bypass,
                                            OP.mult, accum_out=acc(iT(2), t))
                vector.scalar_tensor_tensor(jD[:], x, 1.0, Yb[b][:], OP.bypass,
                                            OP.mult, accum_out=acc(iU(1), t))
                vector.wait_ge(gD_s, t + 1)
                vector.tensor_scalar(jD[:], jP1[b][:], 1.0, 0.0, OP.mult, OP.add,
                                     accum_out=acc(iS(5), t))
                vector.tensor_scalar(jD[:], jP3[b][:], 1.0, 0.0, OP.mult, OP.add,
                                     accum_out=acc(iU(2), t)).then_inc(dD_s, 1)

            # ---------------- tail on DVE ----------------
            vector.wait_ge(aD_s, NT)
            vector.wait_ge(gD_s, NT)

            algv = alg[:]
            slot = [0]

            def T():
                s = slot[0]
                assert s < NALG
                slot[0] += 1
                return algv[:, s * NT : (s + 1) * NT]

            def cwj(j):
                return csbv[:, :, j]   # [P, NT] stride 6

            s0 = T()
            vector.memset(s0, float(N))

            def S(k):
                return s0 if k == 0 else mom(iS(k))

            A = [[T() for _ in range(5)] for _ in range(4)]
            for i in range(4):
                for j in range(4):
                    vector.tensor_copy(A[i][j], S((3 - i) + (3 - j)))
                vector.tensor_copy(A[i][4], mom(iT(3 - i)))

            for k in range(4):
                piv = T()
                vector.reciprocal(piv, A[k][k])
                for i in range(k + 1, 4):
                    f = T()
                    vector.tensor_mul(f, A[i][k], piv)
                    for j in range(k + 1, 5):
                        tmp = T()
                        vector.tensor_mul(tmp, f, A[k][j])
                        vector.tensor_sub(A[i][j], A[i][j], tmp)

            w = [None] * 4
            for i in range(3, -1, -1):
                accv = T()
                vector.tensor_copy(accv, A[i][4])
                for j in range(i + 1, 4):
                    tmp = T()
                    vector.tensor_mul(tmp, A[i][j], w[j])
                    vector.tensor_sub(accv, accv, tmp)
                piv = T()
                vector.reciprocal(piv, A[i][i])
                wi = T()
                vector.tensor_mul(wi, accv, piv)
                w[i] = wi

            a0 = T()
            vector.reciprocal(a0, cwj(0))
            dt_ = T()
            vector.tensor_mul(dt_, cwj(4), cwj(5))
            vector.tensor_sub(dt_, cwj(3), dt_)
            vector.tensor_mul(dt_, cwj(0), dt_)
            rdet = T()
            vector.reciprocal(rdet, dt_)
            a1 = T()
            vector.tensor_mul(a1, cwj(2), cwj(5))
            vector.tensor_sub(a1, a1, cwj(1))
            vector.tensor_mul(a1, a1, rdet)
            a2 = T()
            t_a = T()
            vector.tensor_mul(a2, cwj(1), cwj(4))
            vector.tensor_mul(t_a, cwj(2), cwj(3))
            vector.tensor_sub(a2, a2, t_a)
            vector.tensor_mul(a2, a2, rdet)

            p = [T() for _ in range(4)]
            vector.tensor_mul(p[0], a0, w[0])
            vector.tensor_mul(p[1], a0, w[1])
            vector.tensor_mul(p[2], a0, w[2])
            vector.tensor_add(p[2], p[2], a1)
            vector.tensor_mul(p[3], a0, w[3])
            vector.tensor_add(p[3], p[3], a2)

            Uv = [mom(iU(3)), mom(iU(2)), mom(iU(1)), mom(iU(0))]
            pU = T()
            vector.tensor_mul(pU, p[0], Uv[0])
            for i in range(1, 4):
                tmp = T()
                vector.tensor_mul(tmp, p[i], Uv[i])
                vector.tensor_add(pU, pU, tmp)

            pGp = T()
            for i in range(4):
                gp = T()
                vector.tensor_mul(gp, S((3 - i) + 3), p[0])
                for j in range(1, 4):
                    tmp = T()
                    vector.tensor_mul(tmp, S((3 - i) + (3 - j)), p[j])
                    vector.tensor_add(gp, gp, tmp)
                vector.tensor_mul(gp, gp, p[i])
                if i == 0:
                    vector.tensor_copy(pGp, gp)
                else:
                    vector.tensor_add(pGp, pGp, gp)

            loss = T()
            vector.tensor_scalar_mul(loss, pU, -2.0)
            vector.tensor_add(loss, loss, mom(iV))
            vector.tensor_add(loss, loss, pGp)

            # NaN-force: non-finite samples must yield NaN (not +/-inf)
            nanfix = T()
            vector.tensor_sub(nanfix, loss, loss)
            vector.tensor_add(loss, loss, nanfix)

            vector.tensor_reduce(red[:], loss, AX.X, OP.add).then_inc(vT_s, 1)

    return nc


_NC_CACHE = None


def _get_nc():
    global _NC_CACHE
    if _NC_CACHE is None:
        _NC_CACHE = _build_kernel()
    return _NC_CACHE


def kernel(gt_pts: np.ndarray, transformation_coefficients: np.ndarray) -> np.ndarray:
    gt = np.ascontiguousarray(gt_pts, dtype=np.float32)
    co = np.ascontiguousarray(transformation_coefficients, dtype=np.float32)
    nc = _get_nc()
    in_maps = [
        {
            "gt_pts": gt[i * BS : (i + 1) * BS],
            "transformation_coefficients": co[i * BS : (i + 1) * BS],
        }
        for i in range(NCORES)
    ]
    res = run_bass_kernel_spmd(nc, in_maps, core_ids=list(range(NCORES)))
    total = np.float32(0.0)
    for r in res.results:
        total = np.float32(total + np.sum(r["loss_part"], dtype=np.float32))
    return np.asarray(total / np.float32(B * N), dtype=np.float32)
